# revision 9
# baseline (speedup 1.0000x reference)
"""Multi-head causal attention (B=2, S=2048, D=1024, H=16) on 8 trn2 NeuronCores.

Strategy (tensor-parallel over heads, per the sharding hint):
  - Each core owns 2 heads (128 of 1024 hidden dims): W_q/W_k/W_v column-parallel.
  - Activations kept transposed ([dim, token]) end to end so every matmul
    contracts on the partition axis with zero on-device transposes of x.
  - Fully software-pipelined: for each 512-token tile, project Q/K/V,
    transpose V, then run that q-tile's causal attention - the PE never waits
    for a separate projection phase.
  - scores^T = K^T.T @ Q^T per 128-key-chunk x 512-query-tile, two heads packed
    into disjoint PE row-groups (contraction is only dk=64).
  - softmax without max-subtraction (scores are O(1)); rowsum folded into the
    PV matmul via an augmented V [keys, 64+1] whose last column is ones.
  - exp only on the causal part of diagonal chunks; the rest of the P tile is
    zeroed, and only the 128-wide diagonal strip is tri-masked.
  - normalization fed straight off the PV PSUM rowsum row: per-head DVE
    reciprocal -> gpsimd partition-broadcast -> one fused [128,512] multiply.
  - q-tiles processed batch-interleaved (b0j0, b1j0, b0j1, ...) and ctx
    re-sharded token-parallel with FOUR AllToAlls (one per half-batch), each
    issued as soon as its two q-tiles finish; ctx loads + out-projection for
    each quarter run under later attention, so only the last small a2a plus
    one 128-token out-projection is exposed at the tail.
  - out-projection runs with full W_o on each core for its 4x128 tokens.
  - bf16 matmul inputs everywhere; PSUM accumulation and softmax
    normalization stay fp32.

kernel(**inputs) takes the full unsharded inputs and returns the full output.
"""

import numpy as np
import ml_dtypes

import concourse.bass as bass
import concourse.mybir as mybir
import concourse.tile as tile
from concourse import bacc
from concourse.bass import ts
from concourse.bass_utils import run_bass_kernel_spmd

B, S, D = 2, 2048, 1024
H, DK = 16, 64
NCORE = 8
T = B * S          # 4096 tokens
TT = 512           # token tile (projections, q-tiles)
NT = T // TT       # 8
KC = 128           # key chunk
NJ = S // TT       # 4 q-tiles per batch
SCALE = 1.0 / np.sqrt(DK)

# batch-interleaved q-tile order; ORDER[i] = (b, j), its token tile is b*NJ+j
ORDER = [(0, 0), (1, 0), (0, 1), (1, 1), (0, 2), (1, 2), (0, 3), (1, 3)]
TILE_OF = [b * NJ + j for (b, j) in ORDER]
# a2a group of q-tile (b, j): k = b + 2*(j//2); group covers 1024 tokens
A_OF = {(b, j): b + 2 * (j // 2) for (b, j) in ORDER}

f32 = mybir.dt.float32
bf16 = mybir.dt.bfloat16
EXP = mybir.ActivationFunctionType.Exp
MULT = mybir.AluOpType.mult
npbf = ml_dtypes.bfloat16


def build_program():
    nc = bacc.Bacc("TRN2", target_bir_lowering=False, debug=False,
                   num_devices=NCORE)

    # tile 0 of x as its own input so its host->device upload lands first
    xT0_d = nc.dram_tensor("xT0", [128, 8, TT], bf16, kind="ExternalInput").ap()
    wT_d = nc.dram_tensor("wT", [128, 8, 3, 128], bf16, kind="ExternalInput").ap()
    xTr_d = nc.dram_tensor("xTr", [NT - 1, 128, 8, TT], bf16,
                           kind="ExternalInput").ap()
    trimask_d = nc.dram_tensor("trimask", [128, 128], bf16, kind="ExternalInput").ap()
    ident_d = nc.dram_tensor("ident", [128, 128], bf16, kind="ExternalInput").ap()
    bqkv_d = nc.dram_tensor("bqkv", [128, 3], f32, kind="ExternalInput").ap()
    bo_d = nc.dram_tensor("bo", [1, 1024], f32, kind="ExternalInput").ap()
    woT_d = nc.dram_tensor("woT", [128, 8, 1024], bf16, kind="ExternalInput").ap()
    outT_d = nc.dram_tensor("outT", [B, 2, 128, 1024], f32, kind="ExternalOutput").ap()

    with tile.TileContext(nc) as tc:
        with (
            tc.tile_pool(name="const", bufs=1) as constp,
            tc.tile_pool(name="wostream", bufs=1) as wop,
            tc.tile_pool(name="xstream", bufs=2) as xp,
            tc.tile_pool(name="qkv", bufs=NT) as qkvp,
            tc.tile_pool(name="vaug", bufs=NJ) as vaugp,
            tc.tile_pool(name="ptile", bufs=4) as pp,
            tc.tile_pool(name="post", bufs=2) as postp,
            tc.tile_pool(name="cxn", bufs=2) as cxnp,
            tc.tile_pool(name="outsb", bufs=2) as outp,
            tc.tile_pool(name="ps_s", bufs=2, space="PSUM") as ps_s,
            tc.tile_pool(name="ps_ctx", bufs=1, space="PSUM") as ps_ctx,
            tc.tile_pool(name="ps_misc", bufs=2, space="PSUM") as ps_misc,
            tc.tile_pool(name="dram", bufs=1, space="DRAM") as dramp,
        ):
            # ---- constants (order = sync-ring order; x0 + wT unblock the PE) ----
            xt0 = xp.tile([128, 8, TT], bf16, tag="xt")
            for o in range(8):
                nc.scalar.dma_start(xt0[:, o, :], xT0_d[:, o, :])
            wT = constp.tile([128, 8, 3, 128], bf16, tag="wT")
            nc.sync.dma_start(wT[:], wT_d)
            ident = constp.tile([128, 128], bf16, tag="ident")
            nc.sync.dma_start(ident[:], ident_d)
            bqkv = constp.tile([128, 3], f32, tag="bqkv")
            nc.sync.dma_start(bqkv[:], bqkv_d)
            trimask = constp.tile([128, 128], bf16, tag="trimask")
            nc.sync.dma_start(trimask[:], trimask_d)

            # W_o / b_o ride the gpsimd ring (off the x-stream path)
            wo_sb = wop.tile([128, 8, 1024], bf16, tag="wo")
            nc.gpsimd.dma_start(wo_sb[:], woT_d)
            bo_row = wop.tile([1, 1024], f32, tag="bor")
            nc.gpsimd.dma_start(bo_row[:], bo_d)
            bo_sb = wop.tile([128, 1024], f32, tag="bobc")
            nc.gpsimd.partition_broadcast(bo_sb[:], bo_row[:], channels=128)

            # per-token-tile Q/K/V (transposed) and per-tile augmented V
            qkv_t = [[None] * NT for _ in range(3)]   # [j][t] -> [128, TT]
            vaug_t = [[[None] * NJ for _ in range(2)] for _ in range(B)]

            # four a2a groups; dst core c <- its 128-token slice of each group
            a2a_in = [dramp.tile([NCORE, 128, 128], bf16, name=f"a2a_in{k}")
                      for k in range(4)]
            a2a_out = [dramp.tile([NCORE, 128, 128], bf16, name=f"a2a_out{k}")
                       for k in range(4)]

            def proj_tile(t):
                if t == 0:
                    xt = xt0
                else:
                    xt = xp.tile([128, 8, TT], bf16, tag="xt")
                    nc.scalar.dma_start(xt[:], xTr_d[t - 1])
                for j in range(3):
                    ps = ps_misc.tile([128, TT], f32, tag="mm")
                    for o in range(8):
                        nc.tensor.matmul(ps[:], wT[:, o, j, :], xt[:, o, :],
                                         start=(o == 0), stop=(o == 7))
                    qt = qkvp.tile([128, TT], bf16, tag=f"qkv{j}",
                                   name=f"qkv{j}_{t}")
                    nc.vector.tensor_scalar_add(qt[:], ps[:], bqkv[:, j:j + 1])
                    qkv_t[j][t] = qt

            def vtrans_tile(t):
                b, tl = t // NJ, t % NJ
                va = [vaugp.tile([128, NJ, DK + 1], bf16, tag=f"va{b}{h}",
                                 name=f"va{b}{h}_{tl}") for h in range(2)]
                for h in range(2):
                    nc.vector.memset(va[h][:, :, DK:DK + 1], 1.0)
                    vaug_t[b][h][tl] = va[h]
                for kt in range(NJ):
                    ps_t = ps_misc.tile([128, TT], bf16, tag="mm")
                    nc.tensor.transpose(ps_t[:, 0:128],
                                        qkv_t[2][t][:, kt * KC:(kt + 1) * KC],
                                        ident[:])
                    for h in range(2):
                        nc.vector.tensor_copy(va[h][:, kt, 0:DK],
                                              ps_t[:, DK * h:DK * h + DK])

            def attention_qtile(b, j):
                nk = 4 * (j + 1)
                pc = [ps_ctx.tile([DK + 1, TT], f32, tag=f"c{h}", name=f"pc{h}")
                      for h in range(2)]

                def emit_pv(p_tile, m):
                    for h in range(2):
                        nc.tensor.matmul(
                            pc[h][:], vaug_t[b][h][m // 4][:, m % 4, :],
                            p_tile[:, TT * h:TT * (h + 1)],
                            start=(m == 0), stop=(m == nk - 1),
                            skip_group_check=True)

                qt = qkv_t[0][b * NJ + j]
                pending = []
                for m in range(nk):
                    kt_tile = qkv_t[1][b * NJ + m // 4]
                    ko = (m % 4) * KC
                    ps = ps_s.tile([128, 2 * TT], f32, tag="s")
                    nc.tensor.matmul(ps[:, 0:TT], kt_tile[0:DK, ko:ko + KC],
                                     qt[0:DK, :],
                                     start=True, stop=True, tile_position=(0, 0))
                    nc.tensor.matmul(ps[:, TT:], kt_tile[DK:128, ko:ko + KC],
                                     qt[DK:128, :],
                                     start=True, stop=True, tile_position=(64, 0))
                    p = pp.tile([128, 2 * TT], bf16, tag="p")
                    r = m - 4 * j
                    if r >= 0:
                        if r > 0:
                            nc.vector.memset(
                                p[:].rearrange("k (h q) -> k h q", h=2)[:, :, 0:KC * r],
                                0.0)
                        nc.scalar.activation(
                            p[:].rearrange("k (h q) -> k h q", h=2)[:, :, KC * r:],
                            ps[:].rearrange("k (h q) -> k h q", h=2)[:, :, KC * r:],
                            EXP, scale=float(SCALE))
                        nc.vector.tensor_tensor(
                            p[:].rearrange("k (h q) -> k h q", h=2)[:, :, KC * r:KC * (r + 1)],
                            p[:].rearrange("k (h q) -> k h q", h=2)[:, :, KC * r:KC * (r + 1)],
                            trimask[:, None, :].to_broadcast([128, 2, 128]), MULT)
                    else:
                        nc.scalar.activation(p[:], ps[:], EXP, scale=float(SCALE))
                    pending.append((p, m))
                    if len(pending) > 2:   # depth-2: PE never waits on a fresh exp
                        emit_pv(*pending.pop(0))
                for pm in pending:
                    emit_pv(*pm)

                # per-q-tile softmax normalization + ship to the a2a buffer.
                # rowsum rows are gathered into a [128, 8] layout so ONE
                # 128-lane DVE reciprocal covers both heads (a [1, 512] recip
                # would run on a single DVE lane at ~3.3us); the tiny gather
                # DMAs ride the idle GpSimd SWDGE channel; the ship DMAs ride
                # the Vector ring so they never delay x-tile loads on Sync.
                rs_g = postp.tile([128, 8], f32, tag="rsg")
                cxs = []
                for h in range(2):
                    rtmp = cxnp.tile([1, TT], f32, tag="rtmp")
                    nc.vector.tensor_copy(rtmp[:], pc[h][DK:DK + 1, :])
                    cx = cxnp.tile([DK, TT], f32, tag="cx")
                    nc.vector.tensor_copy(cx[:], pc[h][0:DK, :])
                    cxs.append(cx)
                    nc.gpsimd.dma_start(rs_g[:, 4 * h:4 * h + 4], rtmp[:])
                rc_g = postp.tile([128, 8], f32, tag="rcg")
                with nc.allow_low_precision(reason="softmax denominator"):
                    nc.vector.reciprocal(rc_g[:], rs_g[:])
                k = A_OF[(b, j)]
                for h in range(2):
                    rrow = cxnp.tile([1, TT], f32, tag="rrow")
                    nc.gpsimd.dma_start(rrow[:], rc_g[:, 4 * h:4 * h + 4])
                    bcast = cxnp.tile([DK, TT], f32, tag="bcast")
                    nc.gpsimd.partition_broadcast(bcast[:], rrow[:], channels=DK)
                    cxn = cxnp.tile([DK, TT], bf16, tag="cxn")
                    nc.vector.tensor_tensor(cxn[:], cxs[h][:], bcast[:], MULT)
                    for g in range(4):   # 128-token slices -> dst cores 4*(j%2)+g
                        nc.sync.dma_start(
                            a2a_in[k][4 * (j % 2) + g, DK * h:DK * (h + 1), :],
                            cxn[:, KC * g:KC * (g + 1)])

            def do_a2a(k):
                nc.gpsimd.collective_compute(
                    "AllToAll", mybir.AluOpType.bypass,
                    replica_groups=[list(range(NCORE))],
                    ins=[a2a_in[k][:].opt()], outs=[a2a_out[k][:].opt()])

            ctx_tiles = {}

            def load_ctx(k):
                # ctx loads sit on the Sync ring gated on collective k; they
                # are emitted only once nothing urgent remains behind them.
                ctx_sb = constp.tile([128, 8, 128], bf16, tag=f"ctx{k}",
                                     name=f"ctx{k}")
                for d in range(8):
                    nc.sync.dma_start(ctx_sb[:, d, :], a2a_out[k][d])
                ctx_tiles[k] = ctx_sb

            def outproj_quarter(k):
                bk, half = k % 2, k // 2
                ctx_sb = ctx_tiles[k]
                for oh in range(2):      # 512-wide od halves
                    ps = ps_misc.tile([128, TT], f32, tag="mm")
                    for d in range(8):
                        nc.tensor.matmul(
                            ps[:], ctx_sb[:, d, :],
                            wo_sb[:, d, TT * oh:TT * (oh + 1)],
                            start=(d == 0), stop=(d == 7))
                    ot = outp.tile([128, TT], f32, tag="ot")
                    nc.vector.tensor_tensor(
                        ot[:], ps[:], bo_sb[:, TT * oh:TT * (oh + 1)],
                        mybir.AluOpType.add)
                    nc.scalar.dma_start(
                        outT_d[bk, half, :, TT * oh:TT * (oh + 1)], ot[:])

            # ---- pipelined schedule (projection one tile ahead, interleaved
            # batches; a2a fires per half-batch, dest work overlaps attention)
            proj_tile(TILE_OF[0])
            for i in range(NT):
                if i + 1 < NT:
                    proj_tile(TILE_OF[i + 1])
                vtrans_tile(TILE_OF[i])
                attention_qtile(*ORDER[i])
                if i == 2:
                    do_a2a(0)
                elif i == 3:
                    do_a2a(1)
                elif i == 6:
                    # last x-tile DMA is on the Sync ring by now; a2a(0)/(1)
                    # have long completed, so these loads never stall the ring
                    do_a2a(2)
                    load_ctx(0)
                    load_ctx(1)
            # all out-projections go AFTER the last attention tile and the
            # final a2a trigger: quarters 0-2 keep the PE busy while a2a(3)
            # is in flight, so only ctx(3) + one quarter is exposed.
            do_a2a(3)
            load_ctx(2)
            load_ctx(3)
            outproj_quarter(0)
            outproj_quarter(1)
            outproj_quarter(2)
            outproj_quarter(3)

    nc.compile()
    return nc


def make_in_maps(x, Wq, bq, Wk, bk, Wv, bv, Wo, bo):
    x = np.asarray(x, np.float32)
    xT = np.ascontiguousarray(x.reshape(T, D).T)                  # [D, T]
    # [NT, 128, 8, TT]: xT_t[t, p, o, q] = xT[o*128+p, t*TT+q]
    xT_t = np.ascontiguousarray(
        xT.reshape(8, 128, NT, TT).transpose(2, 1, 0, 3)).astype(npbf)

    woT = np.ascontiguousarray(
        np.asarray(Wo, np.float32).T.reshape(8, 128, 1024)
        .transpose(1, 0, 2)).astype(npbf)
    bo_row = np.ascontiguousarray(np.asarray(bo, np.float32)[None, :])

    trimask = (np.arange(128)[:, None] <= np.arange(128)[None, :]).astype(npbf)
    ident = np.eye(128, dtype=npbf)

    in_maps = []
    for c in range(NCORE):
        sl = slice(128 * c, 128 * (c + 1))
        wT_c = np.stack(
            [np.ascontiguousarray(
                np.asarray(W, np.float32)[sl, :].T.reshape(8, 128, 128)
                .transpose(1, 0, 2))
             for W in (Wq, Wk, Wv)], axis=2)                       # [128, 8, 3, 128]
        bqkv_c = np.stack([np.asarray(b_, np.float32)[sl]
                           for b_ in (bq, bk, bv)], axis=1)        # [128, 3]
        in_maps.append({
            "xT0": np.ascontiguousarray(xT_t[0]),
            "xTr": np.ascontiguousarray(xT_t[1:]),
            "wT": np.ascontiguousarray(wT_c).astype(npbf),
            "woT": woT,
            "bqkv": np.ascontiguousarray(bqkv_c),
            "bo": bo_row,
            "trimask": trimask,
            "ident": ident,
        })
    return in_maps


def assemble_output(results):
    # results[c]["outT"]: [B, 2, 128, 1024] = out[(b, 1024*half + 128c + t), od]
    out = np.empty((B, S, D), np.float32)
    for c in range(NCORE):
        for half in range(2):
            out[:, 1024 * half + 128 * c:1024 * half + 128 * (c + 1), :] = \
                results[c]["outT"][:, half]
    return out


_PROGRAM = None


def get_program():
    global _PROGRAM
    if _PROGRAM is None:
        _PROGRAM = build_program()
    return _PROGRAM


def run(in_maps, **kwargs):
    nc = get_program()
    return run_bass_kernel_spmd(nc, in_maps, core_ids=list(range(NCORE)), **kwargs)


def kernel(x, Wq, bq, Wk, bk, Wv, bv, Wo, bo):
    in_maps = make_in_maps(x, Wq, bq, Wk, bk, Wv, bv, Wo, bo)
    res = run(in_maps)
    return assemble_output(res.results)


if __name__ == "__main__":
    rng = np.random.default_rng(0)
    x = rng.standard_normal((B, S, D), dtype=np.float32)
    mk = lambda *s: ((rng.random(s).astype(np.float32)) - 0.5) / 16
    out = kernel(x, mk(D, D), mk(D), mk(D, D), mk(D), mk(D, D), mk(D),
                 mk(D, D), mk(D))
    print(out.shape, out.dtype, np.abs(out).mean())


# revision 13
# speedup vs baseline: 1.2086x; 1.2086x over previous
"""Multi-head causal attention (B=2, S=2048, D=1024, H=16) on 8 trn2 NeuronCores.

Strategy (tensor-parallel over heads, per the sharding hint):
  - Each core owns 2 heads (128 of 1024 hidden dims): W_q/W_k/W_v column-parallel.
  - Activations kept transposed ([dim, token]) end to end so every matmul
    contracts on the partition axis with zero on-device transposes of x.
  - Fully software-pipelined: for each 512-token tile, project Q/K/V,
    transpose V, then run that q-tile's causal attention - the PE never waits
    for a separate projection phase.
  - scores^T = K^T.T @ Q^T per 128-key-chunk x 512-query-tile, two heads packed
    into disjoint PE row-groups (contraction is only dk=64).
  - softmax without max-subtraction (scores are O(1)); rowsum folded into the
    PV matmul via an augmented V [keys, 64+1] whose last column is ones.
  - exp only on the causal part of diagonal chunks; the rest of the P tile is
    zeroed, and only the 128-wide diagonal strip is tri-masked.
  - normalization fed straight off the PV PSUM rowsum row: per-head DVE
    reciprocal -> gpsimd partition-broadcast -> one fused [128,512] multiply.
  - q-tiles processed batch-interleaved (b0j0, b1j0, b0j1, ...) and ctx
    re-sharded token-parallel with FOUR AllToAlls (one per half-batch), each
    issued as soon as its two q-tiles finish; ctx loads + out-projection for
    each quarter run under later attention, so only the last small a2a plus
    one 128-token out-projection is exposed at the tail.
  - out-projection runs with full W_o on each core for its 4x128 tokens.
  - bf16 matmul inputs everywhere; PSUM accumulation and softmax
    normalization stay fp32.

kernel(**inputs) takes the full unsharded inputs and returns the full output.
"""

import numpy as np
import ml_dtypes

import concourse.bass as bass
import concourse.mybir as mybir
import concourse.tile as tile
from concourse import bacc
from concourse.bass import ts
from concourse.bass_utils import run_bass_kernel_spmd

B, S, D = 2, 2048, 1024
H, DK = 16, 64
NCORE = 8
T = B * S          # 4096 tokens
TT = 512           # token tile (projections, q-tiles)
NT = T // TT       # 8
KC = 128           # key chunk
NJ = S // TT       # 4 q-tiles per batch
SCALE = 1.0 / np.sqrt(DK)

# batch-interleaved q-tile order; ORDER[i] = (b, j), its token tile is b*NJ+j
ORDER = [(0, 0), (1, 0), (0, 1), (1, 1), (0, 2), (1, 2), (0, 3), (1, 3)]
TILE_OF = [b * NJ + j for (b, j) in ORDER]
# a2a group of q-tile (b, j): k = b + 2*(j//2); group covers 1024 tokens
A_OF = {(b, j): b + 2 * (j // 2) for (b, j) in ORDER}

f32 = mybir.dt.float32
bf16 = mybir.dt.bfloat16
EXP = mybir.ActivationFunctionType.Exp
MULT = mybir.AluOpType.mult
npbf = ml_dtypes.bfloat16


def build_program():
    nc = bacc.Bacc("TRN2", target_bir_lowering=False, debug=False,
                   num_devices=NCORE)

    # tile 0 of x as its own input so its host->device upload lands first
    xT0_d = nc.dram_tensor("xT0", [128, 8, TT], bf16, kind="ExternalInput").ap()
    wT_d = nc.dram_tensor("wT", [128, 8, 3, 128], bf16, kind="ExternalInput").ap()
    xTr_d = nc.dram_tensor("xTr", [NT - 1, 128, 8, TT], bf16,
                           kind="ExternalInput").ap()
    trimask_d = nc.dram_tensor("trimask", [128, 128], bf16, kind="ExternalInput").ap()
    ident_d = nc.dram_tensor("ident", [128, 128], bf16, kind="ExternalInput").ap()
    bqkv_d = nc.dram_tensor("bqkv", [128, 3], f32, kind="ExternalInput").ap()
    bo_d = nc.dram_tensor("bo", [1, 1024], f32, kind="ExternalInput").ap()
    woT_d = nc.dram_tensor("woT", [128, 8, 1024], bf16, kind="ExternalInput").ap()
    outT_d = nc.dram_tensor("outT", [B, 2, 128, 1024], f32, kind="ExternalOutput").ap()

    with tile.TileContext(nc) as tc:
        with (
            tc.tile_pool(name="const", bufs=1) as constp,
            tc.tile_pool(name="wostream", bufs=1) as wop,
            tc.tile_pool(name="xstream", bufs=2) as xp,
            tc.tile_pool(name="qkv", bufs=NT) as qkvp,
            tc.tile_pool(name="vaug", bufs=NJ) as vaugp,
            tc.tile_pool(name="ptile", bufs=4) as pp,
            tc.tile_pool(name="post", bufs=2) as postp,
            tc.tile_pool(name="cxn", bufs=2) as cxnp,
            tc.tile_pool(name="outsb", bufs=2) as outp,
            tc.tile_pool(name="ps_s", bufs=2, space="PSUM") as ps_s,
            tc.tile_pool(name="ps_ctx", bufs=1, space="PSUM") as ps_ctx,
            tc.tile_pool(name="ps_misc", bufs=2, space="PSUM") as ps_misc,
            tc.tile_pool(name="dram", bufs=1, space="DRAM") as dramp,
        ):
            # ---- constants (order = sync-ring order; x0 + wT unblock the PE) ----
            xt0 = xp.tile([128, 8, TT], bf16, tag="xt")
            for o in range(8):
                nc.scalar.dma_start(xt0[:, o, :], xT0_d[:, o, :])
            wT = constp.tile([128, 8, 3, 128], bf16, tag="wT")
            nc.sync.dma_start(wT[:], wT_d)
            ident = constp.tile([128, 128], bf16, tag="ident")
            nc.sync.dma_start(ident[:], ident_d)
            bqkv = constp.tile([128, 3], f32, tag="bqkv")
            nc.sync.dma_start(bqkv[:], bqkv_d)
            trimask = constp.tile([128, 128], bf16, tag="trimask")
            nc.sync.dma_start(trimask[:], trimask_d)

            # W_o / b_o ride the gpsimd ring (off the x-stream path)
            wo_sb = wop.tile([128, 8, 1024], bf16, tag="wo")
            nc.gpsimd.dma_start(wo_sb[:], woT_d)
            bo_row = wop.tile([1, 1024], f32, tag="bor")
            nc.gpsimd.dma_start(bo_row[:], bo_d)
            bo_sb = wop.tile([128, 1024], f32, tag="bobc")
            nc.gpsimd.partition_broadcast(bo_sb[:], bo_row[:], channels=128)

            # per-token-tile Q/K/V (transposed) and per-tile augmented V
            qkv_t = [[None] * NT for _ in range(3)]   # [j][t] -> [128, TT]
            vaug_t = [[[None] * NJ for _ in range(2)] for _ in range(B)]

            # four a2a groups; dst core c <- its 128-token slice of each group
            a2a_in = [dramp.tile([NCORE, 128, 128], bf16, name=f"a2a_in{k}")
                      for k in range(4)]
            a2a_out = [dramp.tile([NCORE, 128, 128], bf16, name=f"a2a_out{k}")
                       for k in range(4)]

            ones_c = constp.tile([1, DK], bf16, tag="ones_c")
            nc.vector.memset(ones_c[:], 1.0)

            def proj_tile(t):
                if t == 0:
                    xt = xt0
                else:
                    xt = xp.tile([128, 8, TT], bf16, tag="xt")
                    nc.scalar.dma_start(xt[:], xTr_d[t - 1])
                for j in range(3):
                    ps = ps_misc.tile([128, TT], f32, tag="mm")
                    for o in range(8):
                        nc.tensor.matmul(ps[:], wT[:, o, j, :], xt[:, o, :],
                                         start=(o == 0), stop=(o == 7))
                    qt = qkvp.tile([128, TT], bf16, tag=f"qkv{j}",
                                   name=f"qkv{j}_{t}")
                    nc.vector.tensor_scalar_add(qt[:], ps[:], bqkv[:, j:j + 1])
                    qkv_t[j][t] = qt

            def vtrans_tile(t):
                b, tl = t // NJ, t % NJ
                va = [vaugp.tile([128, NJ, DK + 1], bf16, tag=f"va{b}{h}",
                                 name=f"va{b}{h}_{tl}") for h in range(2)]
                for h in range(2):
                    nc.vector.memset(va[h][:, :, DK:DK + 1], 1.0)
                    vaug_t[b][h][tl] = va[h]
                for kt in range(NJ):
                    ps_t = ps_misc.tile([128, TT], bf16, tag="mm")
                    nc.tensor.transpose(ps_t[:, 0:128],
                                        qkv_t[2][t][:, kt * KC:(kt + 1) * KC],
                                        ident[:])
                    for h in range(2):
                        nc.vector.tensor_copy(va[h][:, kt, 0:DK],
                                              ps_t[:, DK * h:DK * h + DK])

            def attention_qtile(b, j):
                nk = 4 * (j + 1)
                pc = [ps_ctx.tile([DK + 1, TT], f32, tag=f"c{h}", name=f"pc{h}")
                      for h in range(2)]

                def emit_pv(p_tile, m):
                    for h in range(2):
                        nc.tensor.matmul(
                            pc[h][:], vaug_t[b][h][m // 4][:, m % 4, :],
                            p_tile[:, TT * h:TT * (h + 1)],
                            start=(m == 0), stop=(m == nk - 1),
                            skip_group_check=True)

                qt = qkv_t[0][b * NJ + j]
                pending = []
                for m in range(nk):
                    kt_tile = qkv_t[1][b * NJ + m // 4]
                    ko = (m % 4) * KC
                    ps = ps_s.tile([128, 2 * TT], f32, tag="s")
                    nc.tensor.matmul(ps[:, 0:TT], kt_tile[0:DK, ko:ko + KC],
                                     qt[0:DK, :],
                                     start=True, stop=True, tile_position=(0, 0))
                    nc.tensor.matmul(ps[:, TT:], kt_tile[DK:128, ko:ko + KC],
                                     qt[DK:128, :],
                                     start=True, stop=True, tile_position=(64, 0))
                    p = pp.tile([128, 2 * TT], bf16, tag="p")
                    r = m - 4 * j
                    if r >= 0:
                        if r > 0:
                            nc.vector.memset(
                                p[:].rearrange("k (h q) -> k h q", h=2)[:, :, 0:KC * r],
                                0.0)
                        nc.scalar.activation(
                            p[:].rearrange("k (h q) -> k h q", h=2)[:, :, KC * r:],
                            ps[:].rearrange("k (h q) -> k h q", h=2)[:, :, KC * r:],
                            EXP, scale=float(SCALE))
                        nc.vector.tensor_tensor(
                            p[:].rearrange("k (h q) -> k h q", h=2)[:, :, KC * r:KC * (r + 1)],
                            p[:].rearrange("k (h q) -> k h q", h=2)[:, :, KC * r:KC * (r + 1)],
                            trimask[:, None, :].to_broadcast([128, 2, 128]), MULT)
                    else:
                        nc.scalar.activation(p[:], ps[:], EXP, scale=float(SCALE))
                    pending.append((p, m))
                    if len(pending) > 2:   # depth-2: PE never waits on a fresh exp
                        emit_pv(*pending.pop(0))
                for pm in pending:
                    emit_pv(*pm)

                # per-q-tile softmax normalization, phase 1 (phase 2 - the
                # broadcast + multiply + ship - is deferred into the NEXT
                # iteration via finish_norm so the PE never waits on this
                # chain). Rowsum rows are gathered into a [128, 8] layout so
                # ONE 128-lane DVE reciprocal covers both heads (a [1, 512]
                # recip would run on a single DVE lane at ~3.3us); the tiny
                # gather/scatter DMAs ride the Sync ring, keeping the GpSimd
                # ring free for collective triggers only.
                rs_g = postp.tile([128, 8], f32, tag="rsg")
                cxs = []
                for h in range(2):
                    rtmp = cxnp.tile([1, TT], f32, tag="rtmp")
                    nc.vector.tensor_copy(rtmp[:], pc[h][DK:DK + 1, :])
                    cx = cxnp.tile([DK, TT], f32, tag="cx")
                    nc.vector.tensor_copy(cx[:], pc[h][0:DK, :])
                    cxs.append(cx)
                    nc.sync.dma_start(rs_g[:, 4 * h:4 * h + 4], rtmp[:])
                rc_g = postp.tile([128, 8], bf16, tag="rcg")
                with nc.allow_low_precision(reason="softmax denominator"):
                    nc.vector.reciprocal(rc_g[:], rs_g[:])
                rrow = cxnp.tile([1, 2, TT], bf16, tag="rrow")
                for h in range(2):
                    nc.sync.dma_start(rrow[:, h, :], rc_g[:, 4 * h:4 * h + 4])
                return {"cxs": cxs, "rrow": rrow, "b": b, "j": j}

            def finish_norm(st):
                # phase 2: PE outer-product broadcasts the reciprocal rows
                # (ones[1,64] x rrow[1,512] into PSUM row-groups 0-63/64-127),
                # then one multiply per head and the ship DMAs to a2a_in.
                b, j = st["b"], st["j"]
                k = A_OF[(b, j)]
                bc_ps = ps_misc.tile([128, TT], f32, tag="mm")
                for h in range(2):
                    nc.tensor.matmul(bc_ps[DK * h:DK * (h + 1), :], ones_c[:],
                                     st["rrow"][:, h, :], start=True, stop=True,
                                     tile_position=(0, DK * h))
                for h in range(2):
                    cxn = cxnp.tile([DK, TT], bf16, tag="cxn")
                    nc.vector.tensor_tensor(cxn[:], st["cxs"][h][:],
                                            bc_ps[DK * h:DK * (h + 1), :], MULT)
                    for g in range(4):   # 128-token slices -> dst cores 4*(j%2)+g
                        nc.sync.dma_start(
                            a2a_in[k][4 * (j % 2) + g, DK * h:DK * (h + 1), :],
                            cxn[:, KC * g:KC * (g + 1)])

            def do_a2a(k):
                nc.gpsimd.collective_compute(
                    "AllToAll", mybir.AluOpType.bypass,
                    replica_groups=[list(range(NCORE))],
                    ins=[a2a_in[k][:].opt()], outs=[a2a_out[k][:].opt()])

            ctx_tiles = {}

            def load_ctx(k):
                # ctx loads sit on the Sync ring gated on collective k; they
                # are emitted only once nothing urgent remains behind them.
                ctx_sb = constp.tile([128, 8, 128], bf16, tag=f"ctx{k}",
                                     name=f"ctx{k}")
                for d in range(8):
                    nc.sync.dma_start(ctx_sb[:, d, :], a2a_out[k][d])
                ctx_tiles[k] = ctx_sb

            def outproj_quarter(k):
                bk, half = k % 2, k // 2
                ctx_sb = ctx_tiles[k]
                for oh in range(2):      # 512-wide od halves
                    ps = ps_misc.tile([128, TT], f32, tag="mm")
                    for d in range(8):
                        nc.tensor.matmul(
                            ps[:], ctx_sb[:, d, :],
                            wo_sb[:, d, TT * oh:TT * (oh + 1)],
                            start=(d == 0), stop=(d == 7))
                    ot = outp.tile([128, TT], f32, tag="ot")
                    nc.vector.tensor_tensor(
                        ot[:], ps[:], bo_sb[:, TT * oh:TT * (oh + 1)],
                        mybir.AluOpType.add)
                    nc.scalar.dma_start(
                        outT_d[bk, half, :, TT * oh:TT * (oh + 1)], ot[:])

            # ---- pipelined schedule: projection one tile ahead, interleaved
            # batches, normalization finishing one q-tile behind (so the PE
            # outer-product never waits on the reciprocal chain), a2a per
            # half-batch with the gpsimd ring dedicated to the triggers.
            proj_tile(TILE_OF[0])
            norm_st = None
            for i in range(NT):
                if i + 1 < NT:
                    proj_tile(TILE_OF[i + 1])
                if norm_st is not None:
                    finish_norm(norm_st)
                if i == 3:       # ships of q-tiles 0+2 done in iters 1/3
                    do_a2a(0)
                elif i == 4:     # ships of q-tiles 1+3 done in iters 2/4
                    do_a2a(1)
                elif i == 6:
                    # a2a(0)/(1) have long completed; nothing urgent remains
                    # behind these loads on the Sync ring
                    load_ctx(0)
                    load_ctx(1)
                vtrans_tile(TILE_OF[i])
                norm_st = attention_qtile(*ORDER[i])
            do_a2a(2)            # ships of q-tiles 4+6 done in iters 5/7
            # outproj(0) fills the PE while the last norm chain drains
            outproj_quarter(0)
            finish_norm(norm_st)
            do_a2a(3)
            load_ctx(2)
            outproj_quarter(1)
            outproj_quarter(2)
            load_ctx(3)
            outproj_quarter(3)

    nc.compile()
    return nc


def make_in_maps(x, Wq, bq, Wk, bk, Wv, bv, Wo, bo):
    x = np.asarray(x, np.float32)
    xT = np.ascontiguousarray(x.reshape(T, D).T)                  # [D, T]
    # [NT, 128, 8, TT]: xT_t[t, p, o, q] = xT[o*128+p, t*TT+q]
    xT_t = np.ascontiguousarray(
        xT.reshape(8, 128, NT, TT).transpose(2, 1, 0, 3)).astype(npbf)

    woT = np.ascontiguousarray(
        np.asarray(Wo, np.float32).T.reshape(8, 128, 1024)
        .transpose(1, 0, 2)).astype(npbf)
    bo_row = np.ascontiguousarray(np.asarray(bo, np.float32)[None, :])

    trimask = (np.arange(128)[:, None] <= np.arange(128)[None, :]).astype(npbf)
    ident = np.eye(128, dtype=npbf)

    in_maps = []
    for c in range(NCORE):
        sl = slice(128 * c, 128 * (c + 1))
        wT_c = np.stack(
            [np.ascontiguousarray(
                np.asarray(W, np.float32)[sl, :].T.reshape(8, 128, 128)
                .transpose(1, 0, 2))
             for W in (Wq, Wk, Wv)], axis=2)                       # [128, 8, 3, 128]
        bqkv_c = np.stack([np.asarray(b_, np.float32)[sl]
                           for b_ in (bq, bk, bv)], axis=1)        # [128, 3]
        in_maps.append({
            "xT0": np.ascontiguousarray(xT_t[0]),
            "xTr": np.ascontiguousarray(xT_t[1:]),
            "wT": np.ascontiguousarray(wT_c).astype(npbf),
            "woT": woT,
            "bqkv": np.ascontiguousarray(bqkv_c),
            "bo": bo_row,
            "trimask": trimask,
            "ident": ident,
        })
    return in_maps


def assemble_output(results):
    # results[c]["outT"]: [B, 2, 128, 1024] = out[(b, 1024*half + 128c + t), od]
    out = np.empty((B, S, D), np.float32)
    for c in range(NCORE):
        for half in range(2):
            out[:, 1024 * half + 128 * c:1024 * half + 128 * (c + 1), :] = \
                results[c]["outT"][:, half]
    return out


_PROGRAM = None


def get_program():
    global _PROGRAM
    if _PROGRAM is None:
        _PROGRAM = build_program()
    return _PROGRAM


def run(in_maps, **kwargs):
    nc = get_program()
    return run_bass_kernel_spmd(nc, in_maps, core_ids=list(range(NCORE)), **kwargs)


def kernel(x, Wq, bq, Wk, bk, Wv, bv, Wo, bo):
    in_maps = make_in_maps(x, Wq, bq, Wk, bk, Wv, bv, Wo, bo)
    res = run(in_maps)
    return assemble_output(res.results)


if __name__ == "__main__":
    rng = np.random.default_rng(0)
    x = rng.standard_normal((B, S, D), dtype=np.float32)
    mk = lambda *s: ((rng.random(s).astype(np.float32)) - 0.5) / 16
    out = kernel(x, mk(D, D), mk(D), mk(D, D), mk(D), mk(D, D), mk(D),
                 mk(D, D), mk(D))
    print(out.shape, out.dtype, np.abs(out).mean())
